# revision 1
# baseline (speedup 1.0000x reference)
import numpy as np
import jax
import jax.numpy as jnp
from functools import partial

# Hardcoded problem shapes (nn_Attention_28321014350079)
B, C, H, W = 16, 128, 56, 56
NUM_HEADS = 4
SR = 4
HK, WK = 14, 14
EPS = 1e-5
N_CORES = 8


def _dwconv(x, w, stride, pad):
    return jax.lax.conv_general_dilated(
        x, w, window_strides=(stride, stride),
        padding=((pad, pad), (pad, pad)),
        dimension_numbers=("NCHW", "OIHW", "NCHW"),
        feature_group_count=x.shape[1])


def _bn_eval(x, g, b):
    return x * (g / np.sqrt(1.0 + EPS)).reshape(1, -1, 1, 1) + b.reshape(1, -1, 1, 1)


def _core_fn(x, rpe, q_w, q_b, kv_w, kv_b, sr1_w, sr1_g, sr1_b,
             sr2_w, sr2_g, sr2_b, lc_w, lc_b):
    # x: [B/8, C, H, W] local batch shard; everything else replicated
    Bl = x.shape[0]
    head_dim = C // NUM_HEADS
    scale = head_dim ** (-0.5)

    q = jnp.einsum("bchw,oc->bohw", x, q_w) + q_b.reshape(1, -1, 1, 1)
    q = q.reshape(Bl, NUM_HEADS, head_dim, H * W).transpose(0, 1, 3, 2)

    h = _dwconv(x, sr1_w, SR, (SR + 3) // 2)
    h = jax.nn.gelu(_bn_eval(h, sr1_g, sr1_b), approximate=False)
    h = _bn_eval(h * sr2_w.reshape(1, -1, 1, 1), sr2_g, sr2_b)

    kv_feat = _dwconv(h, lc_w, 1, 1) + lc_b.reshape(1, -1, 1, 1) + h

    kv = jnp.einsum("bchw,oc->bohw", kv_feat, kv_w) + kv_b.reshape(1, -1, 1, 1)
    k, v = jnp.split(kv, 2, axis=1)
    nk = HK * WK
    k = k.reshape(Bl, NUM_HEADS, head_dim, nk)
    v = v.reshape(Bl, NUM_HEADS, head_dim, nk).transpose(0, 1, 3, 2)

    attn = jnp.einsum("bhqd,bhdk->bhqk", q, k) * scale + rpe
    attn = jax.nn.softmax(attn, axis=-1)

    out = jnp.einsum("bhqk,bhkd->bhqd", attn, v)
    out = out.transpose(0, 1, 3, 2).reshape(Bl, C, H, W)
    return out


_pmapped = None


def _get_pmapped():
    global _pmapped
    if _pmapped is None:
        _pmapped = jax.pmap(
            _core_fn,
            in_axes=(0,) + (None,) * 13,
            devices=jax.devices()[:N_CORES],
        )
    return _pmapped


def kernel(x, rpe, q_w, q_b, kv_w, kv_b, sr1_w, sr1_g, sr1_b,
           sr2_w, sr2_g, sr2_b, lc_w, lc_b):
    # Shard batch across 8 cores: [8, B/8, C, H, W]
    xs = np.ascontiguousarray(np.reshape(np.asarray(x, np.float32),
                                         (N_CORES, B // N_CORES, C, H, W)))
    rpe = np.asarray(rpe, np.float32)
    fn = _get_pmapped()
    out = fn(xs, rpe, np.asarray(q_w, np.float32), np.asarray(q_b, np.float32),
             np.asarray(kv_w, np.float32), np.asarray(kv_b, np.float32),
             np.asarray(sr1_w, np.float32), np.asarray(sr1_g, np.float32),
             np.asarray(sr1_b, np.float32), np.asarray(sr2_w, np.float32),
             np.asarray(sr2_g, np.float32), np.asarray(sr2_b, np.float32),
             np.asarray(lc_w, np.float32), np.asarray(lc_b, np.float32))
    out = np.asarray(out, np.float32).reshape(B, C, H, W)
    return out
